# revision 9
# baseline (speedup 1.0000x reference)
"""Multi-head attention (B=2, S=2048, D=1024, H=16) on 8 TRN2 NeuronCores.

Sharding: tensor-parallel over heads — 2 heads per core. Each core computes
its head group's QKV projections, attention probabilities and attn@V, plus a
partial output projection; the host sums the 8 partial outputs and
concatenates the per-core attention blocks.

Per-core pipeline (all matmuls in fp32r: ~bf16 speed, ~1.5e-4 rel precision):
  - qhT/khT in depth-major layout [128, B*S] straight from the projection
    matmuls; vh token-major with an extra ones-column, so the attn@V matmul
    also produces the softmax denominators for free.
  - scoresT [k,q] per (b, h, q-chunk); ACT exp with no max-subtraction
    (logits for this problem's scale are O(1), exp cannot overflow); attn@V
    accumulated in PSUM over the 16 k-tiles.
  - exp tiles PE-transposed back to [q,k]; normalization fused into the
    PSUM->SBUF eviction as a per-partition tensor_scalar multiply; attention
    rows written with fully contiguous 1MB DMAs.
  - output projection consumes unnormalized attn@V (the 1/denominator row
    scale commutes through the projection) and normalizes on PSUM eviction.
"""

import os
import sys

import numpy as np

for _p in ("/opt/trn_rl_repo",):
    if _p not in sys.path and os.path.isdir(_p):
        sys.path.append(_p)

import concourse.bass as bass  # noqa: E402
import concourse.tile as tile  # noqa: E402
from concourse import bacc, mybir  # noqa: E402
from concourse.bass_utils import run_bass_kernel_spmd  # noqa: E402
from concourse.masks import make_identity  # noqa: E402

F32 = mybir.dt.float32
F32R = mybir.dt.float32r
BF16 = mybir.dt.bfloat16

B, S, D, H = 2, 2048, 1024, 16
DEPTH = D // H          # 64
NCORES = 8
HL = H // NCORES        # 2 heads per core
LOC = HL * DEPTH        # 128 local output dims per core
NTOK = B * S            # 4096
TCH = 512               # token chunk for projections / q chunks
KT = S // 128           # 16 k tiles per batch
QC = S // TCH           # 4 q chunks per batch

_cache = {}
last_exec_ns = None     # set when BASS_ATTN_PROFILE=1


def _build(with_bias_qk, with_bias_v, with_mask):
    nc = bacc.Bacc("TRN2", target_bir_lowering=False, debug=False,
                   num_devices=NCORES)
    ap = {}
    for name, shape, dt in [
        ("xqT", [D, NTOK], F32R), ("xkT", [D, NTOK], F32R),
        ("xvT", [D, NTOK], F32R),
        ("wq", [D, LOC], F32R), ("wk", [D, LOC], F32R), ("wv", [D, LOC], F32R),
        ("wo", [LOC, D], F32R),
    ]:
        ap[name] = nc.dram_tensor(name, shape, dt, kind="ExternalInput").ap()
    if with_bias_qk:
        ap["bq"] = nc.dram_tensor("bq", [LOC, 1], F32, kind="ExternalInput").ap()
        ap["bk"] = nc.dram_tensor("bk", [LOC, 1], F32, kind="ExternalInput").ap()
    if with_bias_v:
        ap["bv"] = nc.dram_tensor("bv", [1, LOC], F32, kind="ExternalInput").ap()
    if with_mask:
        # pre-scaled by -1e9 on host, transposed to [b, k, q]
        ap["maskT"] = nc.dram_tensor("maskT", [B, S, S], F32,
                                     kind="ExternalInput").ap()
        ap["maskQ"] = nc.dram_tensor("maskQ", [B, S, S], F32,
                                     kind="ExternalInput").ap()
    attn_d = nc.dram_tensor("attn", [B, HL, S, S], F32, kind="ExternalOutput").ap()
    out_d = nc.dram_tensor("out", [NTOK, D], F32, kind="ExternalOutput").ap()

    DCH = D // 128  # 8 contraction chunks
    Exp = mybir.ActivationFunctionType.Exp
    Copy = mybir.ActivationFunctionType.Copy
    Ident = mybir.ActivationFunctionType.Identity

    with tile.TileContext(nc) as tc:
        import contextlib
        with contextlib.ExitStack() as ctx:
            const = ctx.enter_context(tc.tile_pool(name="const", bufs=1))
            io = ctx.enter_context(tc.tile_pool(name="io", bufs=10))
            exppool = ctx.enter_context(tc.tile_pool(name="expp", bufs=2))
            rowp = ctx.enter_context(tc.tile_pool(name="rowp", bufs=3))
            denp = ctx.enter_context(tc.tile_pool(name="denp", bufs=2))
            outp = ctx.enter_context(tc.tile_pool(name="outp", bufs=3))
            pp = ctx.enter_context(tc.tile_pool(name="pp", bufs=4, space="PSUM"))
            ppn = ctx.enter_context(tc.tile_pool(name="ppn", bufs=2, space="PSUM"))
            pav = ctx.enter_context(tc.tile_pool(name="pav", bufs=2, space="PSUM"))

            identf = const.tile([32, 32], F32)
            make_identity(nc, identf[:])

            # ---- weights into SBUF ----
            wqt = const.tile([128, DCH, LOC], F32R)
            wkt = const.tile([128, DCH, LOC], F32R)
            wvt = const.tile([128, DCH, LOC], F32R)
            for t, w in ((wqt, ap["wq"]), (wkt, ap["wk"]), (wvt, ap["wv"])):
                nc.sync.dma_start(t[:], w.rearrange("(c p) m -> p c m", p=128))
            wot = const.tile([LOC, D], F32R)
            nc.sync.dma_start(wot[:], ap["wo"])
            if with_bias_qk:
                bq_sb = const.tile([LOC, 1], F32)
                nc.sync.dma_start(bq_sb[:], ap["bq"])
                bk_sb = const.tile([LOC, 1], F32)
                nc.sync.dma_start(bk_sb[:], ap["bk"])
            if with_bias_v:
                bv_row = const.tile([1, LOC], F32)
                nc.sync.dma_start(bv_row[:], ap["bv"])
                bv_sb = const.tile([128, LOC], F32)
                nc.gpsimd.partition_broadcast(bv_sb[:], bv_row[0:1, :])

            # ---- persistent activations ----
            qhT = const.tile([LOC, NTOK], F32R)
            khT = const.tile([LOC, NTOK], F32R)
            vha = const.tile([128, B * KT, 2 * (DEPTH + 1)], BF16)
            aoT = const.tile([LOC, B, S], F32R)

            # ---- q/k projections: dest[c, tok] += w[d, c]^T x[d, tok] ----
            for xname, wt, bias, dest in (
                ("xqT", wqt, "bq", qhT), ("xkT", wkt, "bk", khT),
            ):
                for tcn in range(NTOK // TCH):
                    ps = pp.tile([128, TCH], F32, tag="pp")
                    for dc in range(DCH):
                        xt = io.tile([128, TCH], F32R, tag="x")
                        nc.sync.dma_start(
                            xt[:], ap[xname][dc * 128:(dc + 1) * 128,
                                             tcn * TCH:(tcn + 1) * TCH])
                        nc.tensor.matmul(ps[:], wt[:, dc, :], xt[:],
                                         start=(dc == 0), stop=(dc == DCH - 1))
                    dslice = dest[:, tcn * TCH:(tcn + 1) * TCH]
                    if with_bias_qk:
                        nc.scalar.activation(dslice, ps[:], Ident,
                                             bias=(bq_sb if bias == "bq" else bk_sb)[:])
                    else:
                        nc.scalar.activation(dslice, ps[:], Copy)

            # ---- v projection (token-major) + ones columns ----
            for tt in range(NTOK // 128):
                ps = pp.tile([128, LOC], F32, tag="pp")
                for dc in range(DCH):
                    vt = io.tile([128, 128], F32R, tag="vx")
                    nc.sync.dma_start(
                        vt[:], ap["xvT"][dc * 128:(dc + 1) * 128,
                                         tt * 128:(tt + 1) * 128])
                    nc.tensor.matmul(ps[:], vt[:], wvt[:, dc, :],
                                     start=(dc == 0), stop=(dc == DCH - 1))
                if with_bias_v:
                    nc.vector.tensor_tensor(ps[:], ps[:], bv_sb[:],
                                            op=mybir.AluOpType.add)
                for h in range(HL):
                    nc.scalar.activation(
                        vha[:, tt, h * (DEPTH + 1):h * (DEPTH + 1) + DEPTH],
                        ps[:, h * DEPTH:(h + 1) * DEPTH], Copy)
            for h in range(HL):
                nc.vector.memset(vha[:, :, h * (DEPTH + 1) + DEPTH], 1.0)

            # ---- attention ----
            # per (b, h): stream 1 computes transposed scores [k, q] -> exp
            # (bf16) -> attn@V accumulation (+ softmax denominators via the
            # vh ones-column); stream 2 recomputes scores in natural [q, k]
            # orientation for the attention-probability output, normalized by
            # a per-partition tensor_scalar with the transposed denominators.
            for b in range(B):
                for h in range(HL):
                    hs = slice(h * DEPTH, (h + 1) * DEPTH)
                    vs = slice(h * (DEPTH + 1), (h + 1) * (DEPTH + 1))
                    recip_bh = denp.tile([128, S // 128], F32, tag="recbh")
                    for qc in range(QC):
                        q0 = b * S + qc * TCH
                        expt = exppool.tile([128, KT, TCH], BF16)
                        # dense burst of transposed-score matmuls; exp drains
                        # them to SBUF as PSUM slots free up
                        for kt in range(KT):
                            sc = pp.tile([128, TCH], F32, tag="pp")
                            nc.tensor.matmul(
                                sc[:],
                                khT[hs, b * S + kt * 128:b * S + (kt + 1) * 128],
                                qhT[hs, q0:q0 + TCH], start=True, stop=True)
                            if with_mask:
                                mt = io.tile([128, TCH], F32, tag="x")
                                nc.sync.dma_start(
                                    mt[:].bitcast(F32),
                                    ap["maskT"][b, kt * 128:(kt + 1) * 128,
                                                qc * TCH:(qc + 1) * TCH])
                                nc.vector.tensor_tensor(
                                    sc[:], sc[:], mt[:].bitcast(F32),
                                    op=mybir.AluOpType.add)
                            nc.scalar.activation(expt[:, kt, :], sc[:], Exp)
                        # attn@V accumulation burst (bf16)
                        av = pav.tile([DEPTH + 1, TCH], F32)
                        for kt in range(KT):
                            nc.tensor.matmul(av[:], vha[:, b * KT + kt, vs],
                                             expt[:, kt, :],
                                             start=(kt == 0), stop=(kt == KT - 1),
                                             skip_group_check=True)
                        den = denp.tile([1, TCH], F32)
                        nc.vector.tensor_copy(den[:], av[DEPTH:DEPTH + 1, :])
                        # transposed denominators for the natural-orientation
                        # normalize (per-partition scalars, one per q tile)
                        for qs in range(TCH // 128):
                            qt = qc * (TCH // 128) + qs
                            denT = ppn.tile([128, 1], F32, tag="ppn")
                            nc.tensor.transpose(
                                denT[:], den[0:1, qs * 128:(qs + 1) * 128],
                                identf[0:1, 0:1])
                            nc.vector.reciprocal(recip_bh[:, qt:qt + 1], denT[:])
                        # per-head normalize of attn@V before the output
                        # projection (heads mix in its contraction, so the
                        # 1/denom scale cannot be folded through it)
                        recip_row = denp.tile([1, TCH], F32, tag="rrow")
                        nc.vector.reciprocal(recip_row[:], den[:])
                        rb = denp.tile([DEPTH, TCH], F32, tag="rb")
                        nc.gpsimd.partition_broadcast(rb[:], recip_row[0:1, :])
                        avn = denp.tile([DEPTH, TCH], F32, tag="avn")
                        nc.vector.tensor_tensor(avn[:], av[0:DEPTH, :], rb[:],
                                                op=mybir.AluOpType.mult)
                        nc.scalar.activation(
                            aoT[hs, b, qc * TCH:(qc + 1) * TCH], avn[:], Copy)
                        # natural-orientation attention rows for this q chunk
                        for qs in range(TCH // 128):
                            qt = qc * (TCH // 128) + qs
                            row = rowp.tile([128, S], F32)
                            for kc in range(S // TCH):
                                sc = ppn.tile([128, TCH], F32, tag="ppn")
                                nc.tensor.matmul(
                                    sc[:],
                                    qhT[hs, b * S + qt * 128:b * S + (qt + 1) * 128],
                                    khT[hs, b * S + kc * TCH:b * S + (kc + 1) * TCH],
                                    start=True, stop=True)
                                if with_mask:
                                    mt = io.tile([128, TCH], F32, tag="x")
                                    nc.sync.dma_start(
                                        mt[:].bitcast(F32),
                                        ap["maskQ"][b, qt * 128:(qt + 1) * 128,
                                                    kc * TCH:(kc + 1) * TCH])
                                    nc.vector.tensor_tensor(
                                        sc[:], sc[:], mt[:].bitcast(F32),
                                        op=mybir.AluOpType.add)
                                nc.scalar.activation(
                                    row[:, kc * TCH:(kc + 1) * TCH], sc[:], Exp)
                            nc.vector.tensor_scalar_mul(row[:], row[:],
                                                        recip_bh[:, qt:qt + 1])
                            nc.sync.dma_start(
                                attn_d[b, h, qt * 128:(qt + 1) * 128, :], row[:])

            # ---- output projection ----
            for b in range(B):
                for tt in range(S // 128):
                    for nn in range(D // TCH):
                        ps = pp.tile([128, TCH], F32, tag="pp")
                        nc.tensor.matmul(ps[:], aoT[:, b, tt * 128:(tt + 1) * 128],
                                         wot[:, nn * TCH:(nn + 1) * TCH],
                                         start=True, stop=True)
                        osb = outp.tile([128, TCH], F32)
                        nc.vector.tensor_copy(osb[:], ps[:])
                        nc.sync.dma_start(
                            out_d[b * S + tt * 128:b * S + (tt + 1) * 128,
                                  nn * TCH:(nn + 1) * TCH], osb[:])

    nc.compile()
    return nc


def _profile_shims():
    """Optional NTFF profiling under axon (dev only, BASS_ATTN_PROFILE=1)."""
    import contextlib
    import ctypes
    import types

    so = "/opt/axon/libaxon_pjrt.so"
    try:
        lib = ctypes.CDLL(so)
        if not hasattr(lib, "axon_start_nrt_profile"):
            return False
        lib.axon_start_nrt_profile.argtypes = [ctypes.POINTER(ctypes.c_int64),
                                               ctypes.c_size_t]
        lib.axon_start_nrt_profile.restype = ctypes.c_int64
        lib.axon_stop_nrt_profile.argtypes = [ctypes.c_char_p]
        lib.axon_stop_nrt_profile.restype = ctypes.c_int64

        @contextlib.contextmanager
        def _hook(output_dir, device_ids):
            import jax
            jax.devices()
            if device_ids:
                ids = (ctypes.c_int64 * len(device_ids))(*device_ids)
                rc = lib.axon_start_nrt_profile(ids, len(device_ids))
            else:
                rc = lib.axon_start_nrt_profile(None, 0)
            if rc != 0:
                raise RuntimeError(f"axon_start_nrt_profile rc={rc}")
            try:
                yield
            finally:
                lib.axon_stop_nrt_profile(str(output_dir).encode())

        import antenv
        mod = types.ModuleType("antenv.axon_hooks")
        mod.get_axon_ntff_profile_hook = lambda: _hook
        mod.set_axon_ntff_profile_hook = lambda h: None
        sys.modules["antenv.axon_hooks"] = mod
        antenv.axon_hooks = mod
        from concourse import bass_utils
        bass_utils.upload_artifacts = lambda tmpdir: f"local://{tmpdir}"
        return True
    except OSError:
        return False


def kernel(v, k, q, mask, wq, bq, wk, bk, wv, bv, wo, bo):
    global last_exec_ns
    v, k, q = (np.asarray(x, np.float32) for x in (v, k, q))
    mask = np.asarray(mask, np.float32)
    wq, bq, wk, bk, wv, bv, wo, bo = (
        np.asarray(x, np.float32) for x in (wq, bq, wk, bk, wv, bv, wo, bo))

    scale = 1.0 / np.sqrt(np.float32(DEPTH))
    with_bias_qk = bool(bq.any() or bk.any())
    with_bias_v = bool(bv.any())
    with_mask = bool(mask.any())

    key = (with_bias_qk, with_bias_v, with_mask)
    if key not in _cache:
        _cache[key] = _build(*key)
    nc = _cache[key]

    qT = np.ascontiguousarray(q.reshape(NTOK, D).T)
    kT = np.ascontiguousarray(k.reshape(NTOK, D).T)
    vT = np.ascontiguousarray(v.reshape(NTOK, D).T)

    in_maps = []
    for c in range(NCORES):
        cols = slice(c * LOC, (c + 1) * LOC)
        m = {
            "xqT": qT, "xkT": kT, "xvT": vT,
            # fold the 1/sqrt(depth) score scale into the q projection
            "wq": np.ascontiguousarray(wq[:, cols] * scale),
            "wk": np.ascontiguousarray(wk[:, cols]),
            "wv": np.ascontiguousarray(wv[:, cols]),
            "wo": np.ascontiguousarray(wo[c * LOC:(c + 1) * LOC, :]),
        }
        if with_bias_qk:
            m["bq"] = np.ascontiguousarray((bq[cols] * scale).reshape(LOC, 1))
            m["bk"] = np.ascontiguousarray(bk[cols].reshape(LOC, 1))
        if with_bias_v:
            m["bv"] = np.ascontiguousarray(bv[cols].reshape(1, LOC))
        if with_mask:
            msc = mask[:, 0] * np.float32(-1e9)
            m["maskT"] = np.ascontiguousarray(msc.transpose(0, 2, 1))
            m["maskQ"] = np.ascontiguousarray(msc)
        in_maps.append(m)

    profile = os.environ.get("BASS_ATTN_PROFILE") == "1" and _profile_shims()
    res = run_bass_kernel_spmd(nc, in_maps, list(range(NCORES)),
                               trace=bool(profile))
    last_exec_ns = res.exec_time_ns

    out = np.zeros((NTOK, D), np.float32)
    attn = np.empty((B, H, S, S), np.float32)
    for c in range(NCORES):
        out += res.results[c]["out"]
        attn[:, c * HL:(c + 1) * HL] = res.results[c]["attn"]
    out += bo
    return out.reshape(B, S, D), attn


# revision 11
# speedup vs baseline: 1.0062x; 1.0062x over previous
"""Multi-head attention (B=2, S=2048, D=1024, H=16) on 8 TRN2 NeuronCores.

Sharding: tensor-parallel over heads — 2 heads per core. Each core computes
its head group's QKV projections, attention probabilities and attn@V, plus a
partial output projection; the host sums the 8 partial outputs and
concatenates the per-core attention blocks.

Per-core pipeline (precision-critical matmuls in fp32r; the attn@V pass in
bf16 since it only feeds the output projection):
  - qhT/khT in depth-major layout [128, B*S] straight from the projection
    matmuls; vh token-major with an extra ones-column, so the attn@V matmul
    also produces the softmax denominators for free.
  - scores are computed twice, once per orientation. Transposed [k, q]
    scores -> ACT exp (bf16, no max-subtraction: logits for this problem's
    scale are O(1), exp cannot overflow) -> attn@V accumulated in PSUM.
    Natural [q, k] scores -> ACT exp (f32) feed the attention-probability
    output: per-partition tensor_scalar normalize (denominators transposed
    via tiny PE transposes), then fully contiguous 1MB row DMAs. The dual
    score matmul is much cheaper than transposing 16.8M exp values per core.
  - attn@V is normalized per head (heads mix in the output projection's
    contraction, so the scale can't be folded through it); the output
    projection then runs in fp32r and the host sums the 8 partials.
"""

import os
import sys

import numpy as np

for _p in ("/opt/trn_rl_repo",):
    if _p not in sys.path and os.path.isdir(_p):
        sys.path.append(_p)

import concourse.bass as bass  # noqa: E402
import concourse.tile as tile  # noqa: E402
from concourse import bacc, mybir  # noqa: E402
from concourse.bass_utils import run_bass_kernel_spmd  # noqa: E402
from concourse.masks import make_identity  # noqa: E402

F32 = mybir.dt.float32
F32R = mybir.dt.float32r
BF16 = mybir.dt.bfloat16

B, S, D, H = 2, 2048, 1024, 16
DEPTH = D // H          # 64
NCORES = 8
HL = H // NCORES        # 2 heads per core
LOC = HL * DEPTH        # 128 local output dims per core
NTOK = B * S            # 4096
TCH = 512               # token chunk for projections / q chunks
KT = S // 128           # 16 k tiles per batch
QC = S // TCH           # 4 q chunks per batch

_cache = {}
last_exec_ns = None     # set when BASS_ATTN_PROFILE=1


def _build(with_bias_qk, with_bias_v, with_mask):
    nc = bacc.Bacc("TRN2", target_bir_lowering=False, debug=False,
                   num_devices=NCORES)
    ap = {}
    for name, shape, dt in [
        ("xqT", [D, NTOK], F32R), ("xkT", [D, NTOK], F32R),
        ("xvT", [D, NTOK], F32R),
        ("wq", [D, LOC], F32R), ("wk", [D, LOC], F32R), ("wv", [D, LOC], F32R),
        ("wo", [LOC, D], F32R),
    ]:
        ap[name] = nc.dram_tensor(name, shape, dt, kind="ExternalInput").ap()
    if with_bias_qk:
        ap["bq"] = nc.dram_tensor("bq", [LOC, 1], F32, kind="ExternalInput").ap()
        ap["bk"] = nc.dram_tensor("bk", [LOC, 1], F32, kind="ExternalInput").ap()
    if with_bias_v:
        ap["bv"] = nc.dram_tensor("bv", [1, LOC], F32, kind="ExternalInput").ap()
    if with_mask:
        # pre-scaled by -1e9 on host, transposed to [b, k, q]
        ap["maskT"] = nc.dram_tensor("maskT", [B, S, S], F32,
                                     kind="ExternalInput").ap()
        ap["maskQ"] = nc.dram_tensor("maskQ", [B, S, S], F32,
                                     kind="ExternalInput").ap()
    attn_d = nc.dram_tensor("attn", [B, HL, S, S], F32, kind="ExternalOutput").ap()
    out_d = nc.dram_tensor("out", [NTOK, D], F32, kind="ExternalOutput").ap()

    DCH = D // 128  # 8 contraction chunks
    Exp = mybir.ActivationFunctionType.Exp
    Copy = mybir.ActivationFunctionType.Copy
    Ident = mybir.ActivationFunctionType.Identity

    with tile.TileContext(nc) as tc:
        import contextlib
        with contextlib.ExitStack() as ctx:
            const = ctx.enter_context(tc.tile_pool(name="const", bufs=1))
            io = ctx.enter_context(tc.tile_pool(name="io", bufs=10))
            exppool = ctx.enter_context(tc.tile_pool(name="expp", bufs=2))
            rowp = ctx.enter_context(tc.tile_pool(name="rowp", bufs=3))
            denp = ctx.enter_context(tc.tile_pool(name="denp", bufs=2))
            outp = ctx.enter_context(tc.tile_pool(name="outp", bufs=3))
            pp = ctx.enter_context(tc.tile_pool(name="pp", bufs=4, space="PSUM"))
            ppn = ctx.enter_context(tc.tile_pool(name="ppn", bufs=2, space="PSUM"))
            pav = ctx.enter_context(tc.tile_pool(name="pav", bufs=2, space="PSUM"))

            identf = const.tile([32, 32], F32)
            make_identity(nc, identf[:])

            # ---- weights into SBUF ----
            wqt = const.tile([128, DCH, LOC], F32R)
            wkt = const.tile([128, DCH, LOC], F32R)
            wvt = const.tile([128, DCH, LOC], F32R)
            for t, w in ((wqt, ap["wq"]), (wkt, ap["wk"]), (wvt, ap["wv"])):
                nc.gpsimd.dma_start(t[:], w.rearrange("(c p) m -> p c m", p=128))
            wot = const.tile([LOC, D], F32R)
            nc.sync.dma_start(wot[:], ap["wo"])
            if with_bias_qk:
                bq_sb = const.tile([LOC, 1], F32)
                nc.sync.dma_start(bq_sb[:], ap["bq"])
                bk_sb = const.tile([LOC, 1], F32)
                nc.sync.dma_start(bk_sb[:], ap["bk"])
            if with_bias_v:
                bv_row = const.tile([1, LOC], F32)
                nc.sync.dma_start(bv_row[:], ap["bv"])
                bv_sb = const.tile([128, LOC], F32)
                nc.gpsimd.partition_broadcast(bv_sb[:], bv_row[0:1, :])

            # ---- persistent activations ----
            qhT = const.tile([LOC, NTOK], F32R)
            khT = const.tile([LOC, NTOK], F32R)
            vha = const.tile([128, B * KT, 2 * (DEPTH + 1)], BF16)
            aoT = const.tile([LOC, B, S], F32R)

            # ---- q/k projections: dest[c, tok] += w[d, c]^T x[d, tok] ----
            for xname, wt, bias, dest in (
                ("xqT", wqt, "bq", qhT), ("xkT", wkt, "bk", khT),
            ):
                for tcn in range(NTOK // TCH):
                    ps = pp.tile([128, TCH], F32, tag="pp")
                    for dc in range(DCH):
                        xt = io.tile([128, TCH], F32R, tag="x")
                        nc.gpsimd.dma_start(
                            xt[:], ap[xname][dc * 128:(dc + 1) * 128,
                                             tcn * TCH:(tcn + 1) * TCH])
                        nc.tensor.matmul(ps[:], wt[:, dc, :], xt[:],
                                         start=(dc == 0), stop=(dc == DCH - 1))
                    dslice = dest[:, tcn * TCH:(tcn + 1) * TCH]
                    if with_bias_qk:
                        nc.scalar.activation(dslice, ps[:], Ident,
                                             bias=(bq_sb if bias == "bq" else bk_sb)[:])
                    else:
                        nc.scalar.activation(dslice, ps[:], Copy)

            # ---- v projection (token-major) + ones columns ----
            for tt in range(NTOK // 128):
                ps = pp.tile([128, LOC], F32, tag="pp")
                for dc in range(DCH):
                    vt = io.tile([128, 128], F32R, tag="vx")
                    nc.sync.dma_start(
                        vt[:], ap["xvT"][dc * 128:(dc + 1) * 128,
                                         tt * 128:(tt + 1) * 128])
                    nc.tensor.matmul(ps[:], vt[:], wvt[:, dc, :],
                                     start=(dc == 0), stop=(dc == DCH - 1))
                if with_bias_v:
                    nc.vector.tensor_tensor(ps[:], ps[:], bv_sb[:],
                                            op=mybir.AluOpType.add)
                for h in range(HL):
                    nc.scalar.activation(
                        vha[:, tt, h * (DEPTH + 1):h * (DEPTH + 1) + DEPTH],
                        ps[:, h * DEPTH:(h + 1) * DEPTH], Copy)
            for h in range(HL):
                nc.vector.memset(vha[:, :, h * (DEPTH + 1) + DEPTH], 1.0)

            # ---- attention ----
            # per (b, h): stream 1 computes transposed scores [k, q] -> exp
            # (bf16) -> attn@V accumulation (+ softmax denominators via the
            # vh ones-column); stream 2 recomputes scores in natural [q, k]
            # orientation for the attention-probability output, normalized by
            # a per-partition tensor_scalar with the transposed denominators.
            for b in range(B):
                for h in range(HL):
                    hs = slice(h * DEPTH, (h + 1) * DEPTH)
                    vs = slice(h * (DEPTH + 1), (h + 1) * (DEPTH + 1))
                    recip_bh = denp.tile([128, S // 128], F32, tag="recbh")
                    for qc in range(QC):
                        q0 = b * S + qc * TCH
                        expt = exppool.tile([128, KT, TCH], BF16)
                        # dense burst of transposed-score matmuls; exp drains
                        # them to SBUF as PSUM slots free up
                        for kt in range(KT):
                            sc = pp.tile([128, TCH], F32, tag="pp")
                            nc.tensor.matmul(
                                sc[:],
                                khT[hs, b * S + kt * 128:b * S + (kt + 1) * 128],
                                qhT[hs, q0:q0 + TCH], start=True, stop=True)
                            if with_mask:
                                mt = io.tile([128, TCH], F32, tag="x")
                                nc.sync.dma_start(
                                    mt[:].bitcast(F32),
                                    ap["maskT"][b, kt * 128:(kt + 1) * 128,
                                                qc * TCH:(qc + 1) * TCH])
                                nc.vector.tensor_tensor(
                                    sc[:], sc[:], mt[:].bitcast(F32),
                                    op=mybir.AluOpType.add)
                            nc.scalar.activation(expt[:, kt, :], sc[:], Exp)
                        # attn@V accumulation burst (bf16)
                        av = pav.tile([DEPTH + 1, TCH], F32)
                        for kt in range(KT):
                            nc.tensor.matmul(av[:], vha[:, b * KT + kt, vs],
                                             expt[:, kt, :],
                                             start=(kt == 0), stop=(kt == KT - 1),
                                             skip_group_check=True)
                        den = denp.tile([1, TCH], F32)
                        nc.vector.tensor_copy(den[:], av[DEPTH:DEPTH + 1, :])
                        # transposed denominators for the natural-orientation
                        # normalize (per-partition scalars, one per q tile)
                        for qs in range(TCH // 128):
                            qt = qc * (TCH // 128) + qs
                            denT = ppn.tile([128, 1], F32, tag="ppn")
                            nc.tensor.transpose(
                                denT[:], den[0:1, qs * 128:(qs + 1) * 128],
                                identf[0:1, 0:1])
                            nc.vector.reciprocal(recip_bh[:, qt:qt + 1], denT[:])
                        # per-head normalize of attn@V before the output
                        # projection (heads mix in its contraction, so the
                        # 1/denom scale cannot be folded through it)
                        recip_row = denp.tile([1, TCH], F32, tag="rrow")
                        nc.vector.reciprocal(recip_row[:], den[:])
                        rb = denp.tile([DEPTH, TCH], F32, tag="rb")
                        nc.gpsimd.partition_broadcast(rb[:], recip_row[0:1, :])
                        avn = denp.tile([DEPTH, TCH], F32, tag="avn")
                        nc.vector.tensor_tensor(avn[:], av[0:DEPTH, :], rb[:],
                                                op=mybir.AluOpType.mult)
                        nc.scalar.activation(
                            aoT[hs, b, qc * TCH:(qc + 1) * TCH], avn[:], Copy)
                        # natural-orientation attention rows for this q chunk
                        for qs in range(TCH // 128):
                            qt = qc * (TCH // 128) + qs
                            row = rowp.tile([128, S], F32)
                            for kc in range(S // TCH):
                                sc = ppn.tile([128, TCH], F32, tag="ppn")
                                nc.tensor.matmul(
                                    sc[:],
                                    qhT[hs, b * S + qt * 128:b * S + (qt + 1) * 128],
                                    khT[hs, b * S + kc * TCH:b * S + (kc + 1) * TCH],
                                    start=True, stop=True)
                                if with_mask:
                                    mt = io.tile([128, TCH], F32, tag="x")
                                    nc.sync.dma_start(
                                        mt[:].bitcast(F32),
                                        ap["maskQ"][b, qt * 128:(qt + 1) * 128,
                                                    kc * TCH:(kc + 1) * TCH])
                                    nc.vector.tensor_tensor(
                                        sc[:], sc[:], mt[:].bitcast(F32),
                                        op=mybir.AluOpType.add)
                                nc.scalar.activation(
                                    row[:, kc * TCH:(kc + 1) * TCH], sc[:], Exp)
                            nc.vector.tensor_scalar_mul(row[:], row[:],
                                                        recip_bh[:, qt:qt + 1])
                            nc.sync.dma_start(
                                attn_d[b, h, qt * 128:(qt + 1) * 128, :], row[:])

            # ---- output projection ----
            for b in range(B):
                for tt in range(S // 128):
                    for nn in range(D // TCH):
                        ps = pp.tile([128, TCH], F32, tag="pp")
                        nc.tensor.matmul(ps[:], aoT[:, b, tt * 128:(tt + 1) * 128],
                                         wot[:, nn * TCH:(nn + 1) * TCH],
                                         start=True, stop=True)
                        osb = outp.tile([128, TCH], F32)
                        nc.vector.tensor_copy(osb[:], ps[:])
                        nc.sync.dma_start(
                            out_d[b * S + tt * 128:b * S + (tt + 1) * 128,
                                  nn * TCH:(nn + 1) * TCH], osb[:])

    nc.compile()
    return nc


def _profile_shims():
    """Optional NTFF profiling under axon (dev only, BASS_ATTN_PROFILE=1)."""
    import contextlib
    import ctypes
    import types

    so = "/opt/axon/libaxon_pjrt.so"
    try:
        lib = ctypes.CDLL(so)
        if not hasattr(lib, "axon_start_nrt_profile"):
            return False
        lib.axon_start_nrt_profile.argtypes = [ctypes.POINTER(ctypes.c_int64),
                                               ctypes.c_size_t]
        lib.axon_start_nrt_profile.restype = ctypes.c_int64
        lib.axon_stop_nrt_profile.argtypes = [ctypes.c_char_p]
        lib.axon_stop_nrt_profile.restype = ctypes.c_int64

        @contextlib.contextmanager
        def _hook(output_dir, device_ids):
            import jax
            jax.devices()
            if device_ids:
                ids = (ctypes.c_int64 * len(device_ids))(*device_ids)
                rc = lib.axon_start_nrt_profile(ids, len(device_ids))
            else:
                rc = lib.axon_start_nrt_profile(None, 0)
            if rc != 0:
                raise RuntimeError(f"axon_start_nrt_profile rc={rc}")
            try:
                yield
            finally:
                lib.axon_stop_nrt_profile(str(output_dir).encode())

        import antenv
        mod = types.ModuleType("antenv.axon_hooks")
        mod.get_axon_ntff_profile_hook = lambda: _hook
        mod.set_axon_ntff_profile_hook = lambda h: None
        sys.modules["antenv.axon_hooks"] = mod
        antenv.axon_hooks = mod
        from concourse import bass_utils
        bass_utils.upload_artifacts = lambda tmpdir: f"local://{tmpdir}"
        return True
    except Exception:
        return False


def kernel(v, k, q, mask, wq, bq, wk, bk, wv, bv, wo, bo):
    global last_exec_ns
    v, k, q = (np.asarray(x, np.float32) for x in (v, k, q))
    mask = np.asarray(mask, np.float32)
    wq, bq, wk, bk, wv, bv, wo, bo = (
        np.asarray(x, np.float32) for x in (wq, bq, wk, bk, wv, bv, wo, bo))

    scale = 1.0 / np.sqrt(np.float32(DEPTH))
    with_bias_qk = bool(bq.any() or bk.any())
    with_bias_v = bool(bv.any())
    with_mask = bool(mask.any())

    key = (with_bias_qk, with_bias_v, with_mask)
    if key not in _cache:
        _cache[key] = _build(*key)
    nc = _cache[key]

    qT = np.ascontiguousarray(q.reshape(NTOK, D).T)
    kT = np.ascontiguousarray(k.reshape(NTOK, D).T)
    vT = np.ascontiguousarray(v.reshape(NTOK, D).T)

    in_maps = []
    for c in range(NCORES):
        cols = slice(c * LOC, (c + 1) * LOC)
        m = {
            "xqT": qT, "xkT": kT, "xvT": vT,
            # fold the 1/sqrt(depth) score scale into the q projection
            "wq": np.ascontiguousarray(wq[:, cols] * scale),
            "wk": np.ascontiguousarray(wk[:, cols]),
            "wv": np.ascontiguousarray(wv[:, cols]),
            "wo": np.ascontiguousarray(wo[c * LOC:(c + 1) * LOC, :]),
        }
        if with_bias_qk:
            m["bq"] = np.ascontiguousarray((bq[cols] * scale).reshape(LOC, 1))
            m["bk"] = np.ascontiguousarray(bk[cols].reshape(LOC, 1))
        if with_bias_v:
            m["bv"] = np.ascontiguousarray(bv[cols].reshape(1, LOC))
        if with_mask:
            msc = mask[:, 0] * np.float32(-1e9)
            m["maskT"] = np.ascontiguousarray(msc.transpose(0, 2, 1))
            m["maskQ"] = np.ascontiguousarray(msc)
        in_maps.append(m)

    profile = os.environ.get("BASS_ATTN_PROFILE") == "1" and _profile_shims()
    res = run_bass_kernel_spmd(nc, in_maps, list(range(NCORES)),
                               trace=bool(profile))
    last_exec_ns = res.exec_time_ns

    out = np.zeros((NTOK, D), np.float32)
    attn = np.empty((B, H, S, S), np.float32)
    for c in range(NCORES):
        out += res.results[c]["out"]
        attn[:, c * HL:(c + 1) * HL] = res.results[c]["attn"]
    out += bo
    return out.reshape(B, S, D), attn


# revision 12
# speedup vs baseline: 1.0975x; 1.0907x over previous
"""Multi-head attention (B=2, S=2048, D=1024, H=16) on 8 TRN2 NeuronCores.

Sharding: tensor-parallel over heads — 2 heads per core. Each core computes
its head group's QKV projections, attention probabilities and attn@V, plus a
partial output projection; the host sums the 8 partial outputs and
concatenates the per-core attention blocks.

Per-core pipeline (precision-critical matmuls in fp32r; the attn@V pass in
bf16 since it only feeds the output projection):
  - qhT/khT in depth-major layout [128, B*S] straight from the projection
    matmuls; vh token-major with an extra ones-column, so the attn@V matmul
    also produces the softmax denominators for free.
  - scores are computed twice, once per orientation. Transposed [k, q]
    scores -> ACT exp (bf16, no max-subtraction: logits for this problem's
    scale are O(1), exp cannot overflow) -> attn@V accumulated in PSUM.
    Natural [q, k] scores -> ACT exp (f32) feed the attention-probability
    output: per-partition tensor_scalar normalize (denominators transposed
    via tiny PE transposes), then fully contiguous 1MB row DMAs. The dual
    score matmul is much cheaper than transposing 16.8M exp values per core.
  - attn@V is normalized per head (heads mix in the output projection's
    contraction, so the scale can't be folded through it); the output
    projection then runs in fp32r and the host sums the 8 partials.
"""

import os
import sys

import numpy as np

for _p in ("/opt/trn_rl_repo",):
    if _p not in sys.path and os.path.isdir(_p):
        sys.path.append(_p)

import concourse.bass as bass  # noqa: E402
import concourse.tile as tile  # noqa: E402
from concourse import bacc, mybir  # noqa: E402
from concourse.bass_utils import run_bass_kernel_spmd  # noqa: E402
from concourse.masks import make_identity  # noqa: E402

F32 = mybir.dt.float32
F32R = mybir.dt.float32r
BF16 = mybir.dt.bfloat16

B, S, D, H = 2, 2048, 1024, 16
DEPTH = D // H          # 64
NCORES = 8
HL = H // NCORES        # 2 heads per core
LOC = HL * DEPTH        # 128 local output dims per core
NTOK = B * S            # 4096
TCH = 512               # token chunk for projections / q chunks
KT = S // 128           # 16 k tiles per batch
QC = S // TCH           # 4 q chunks per batch

_cache = {}
last_exec_ns = None     # set when BASS_ATTN_PROFILE=1


def _build(with_bias_qk, with_bias_v, with_mask):
    nc = bacc.Bacc("TRN2", target_bir_lowering=False, debug=False,
                   num_devices=NCORES)
    ap = {}
    for name, shape, dt in [
        ("xqT", [D, NTOK], F32R), ("xkT", [D, NTOK], F32R),
        ("xvT", [D, NTOK], F32R),
        ("wq", [D, LOC], F32R), ("wk", [D, LOC], F32R), ("wv", [D, LOC], F32R),
        ("wo", [LOC, D], F32R),
    ]:
        ap[name] = nc.dram_tensor(name, shape, dt, kind="ExternalInput").ap()
    if with_bias_qk:
        ap["bq"] = nc.dram_tensor("bq", [LOC, 1], F32, kind="ExternalInput").ap()
        ap["bk"] = nc.dram_tensor("bk", [LOC, 1], F32, kind="ExternalInput").ap()
    if with_bias_v:
        ap["bv"] = nc.dram_tensor("bv", [1, LOC], F32, kind="ExternalInput").ap()
    if with_mask:
        # pre-scaled by -1e9 on host, transposed to [b, k, q]
        ap["maskT"] = nc.dram_tensor("maskT", [B, S, S], F32,
                                     kind="ExternalInput").ap()
        ap["maskQ"] = nc.dram_tensor("maskQ", [B, S, S], F32,
                                     kind="ExternalInput").ap()
    attn_d = nc.dram_tensor("attn", [B, HL, S, S], F32, kind="ExternalOutput").ap()
    out_d = nc.dram_tensor("out", [NTOK, D], F32, kind="ExternalOutput").ap()

    DCH = D // 128  # 8 contraction chunks
    Exp = mybir.ActivationFunctionType.Exp
    Copy = mybir.ActivationFunctionType.Copy
    Ident = mybir.ActivationFunctionType.Identity

    with tile.TileContext(nc) as tc:
        import contextlib
        with contextlib.ExitStack() as ctx:
            const = ctx.enter_context(tc.tile_pool(name="const", bufs=1))
            io = ctx.enter_context(tc.tile_pool(name="io", bufs=10))
            exppool = ctx.enter_context(tc.tile_pool(name="expp", bufs=2))
            rowp = ctx.enter_context(tc.tile_pool(name="rowp", bufs=3))
            denp = ctx.enter_context(tc.tile_pool(name="denp", bufs=2))
            outp = ctx.enter_context(tc.tile_pool(name="outp", bufs=3))
            pp = ctx.enter_context(tc.tile_pool(name="pp", bufs=2, space="PSUM"))
            ppn = ctx.enter_context(tc.tile_pool(name="ppn", bufs=3, space="PSUM"))
            pav = ctx.enter_context(tc.tile_pool(name="pav", bufs=1, space="PSUM"))

            identf = const.tile([32, 32], F32)
            make_identity(nc, identf[:])

            # ---- weights into SBUF ----
            wqt = const.tile([128, DCH, LOC], F32R)
            wkt = const.tile([128, DCH, LOC], F32R)
            wvt = const.tile([128, DCH, LOC], F32R)
            for t, w in ((wqt, ap["wq"]), (wkt, ap["wk"]), (wvt, ap["wv"])):
                nc.gpsimd.dma_start(t[:], w.rearrange("(c p) m -> p c m", p=128))
            wot = const.tile([LOC, D], F32R)
            nc.sync.dma_start(wot[:], ap["wo"])
            if with_bias_qk:
                bq_sb = const.tile([LOC, 1], F32)
                nc.sync.dma_start(bq_sb[:], ap["bq"])
                bk_sb = const.tile([LOC, 1], F32)
                nc.sync.dma_start(bk_sb[:], ap["bk"])
            if with_bias_v:
                bv_row = const.tile([1, LOC], F32)
                nc.sync.dma_start(bv_row[:], ap["bv"])
                bv_sb = const.tile([128, LOC], F32)
                nc.gpsimd.partition_broadcast(bv_sb[:], bv_row[0:1, :])

            # ---- persistent activations ----
            qhT = const.tile([LOC, NTOK], F32R)
            khT = const.tile([LOC, NTOK], F32R)
            vha = const.tile([128, B * KT, 2 * (DEPTH + 1)], BF16)
            aoT = const.tile([LOC, B, S], F32R)

            # ---- q/k projections: dest[c, tok] += w[d, c]^T x[d, tok] ----
            for xname, wt, bias, dest in (
                ("xqT", wqt, "bq", qhT), ("xkT", wkt, "bk", khT),
            ):
                for tcn in range(NTOK // TCH):
                    ps2 = pp.tile([128, 2, TCH], F32, tag="pp")
                    ps = ps2[:, 0, :]
                    for dc in range(DCH):
                        xt = io.tile([128, TCH], F32R, tag="x")
                        nc.gpsimd.dma_start(
                            xt[:], ap[xname][dc * 128:(dc + 1) * 128,
                                             tcn * TCH:(tcn + 1) * TCH])
                        nc.tensor.matmul(ps[:], wt[:, dc, :], xt[:],
                                         start=(dc == 0), stop=(dc == DCH - 1))
                    dslice = dest[:, tcn * TCH:(tcn + 1) * TCH]
                    if with_bias_qk:
                        nc.scalar.activation(dslice, ps[:], Ident,
                                             bias=(bq_sb if bias == "bq" else bk_sb)[:])
                    else:
                        nc.scalar.activation(dslice, ps[:], Copy)

            # ---- v projection (token-major) + ones columns ----
            for tt in range(NTOK // 128):
                ps2 = pp.tile([128, 2, LOC], F32, tag="pp")
                ps = ps2[:, 0, :]
                for dc in range(DCH):
                    vt = io.tile([128, 128], F32R, tag="vx")
                    nc.sync.dma_start(
                        vt[:], ap["xvT"][dc * 128:(dc + 1) * 128,
                                         tt * 128:(tt + 1) * 128])
                    nc.tensor.matmul(ps[:], vt[:], wvt[:, dc, :],
                                     start=(dc == 0), stop=(dc == DCH - 1))
                if with_bias_v:
                    nc.vector.tensor_tensor(ps[:], ps[:], bv_sb[:],
                                            op=mybir.AluOpType.add)
                for h in range(HL):
                    nc.scalar.activation(
                        vha[:, tt, h * (DEPTH + 1):h * (DEPTH + 1) + DEPTH],
                        ps[:, h * DEPTH:(h + 1) * DEPTH], Copy)
            for h in range(HL):
                nc.vector.memset(vha[:, :, h * (DEPTH + 1) + DEPTH], 1.0)

            # ---- attention ----
            # per (b, h): stream 1 computes transposed scores [k, q] -> exp
            # (bf16) -> attn@V accumulation (+ softmax denominators via the
            # vh ones-column); stream 2 recomputes scores in natural [q, k]
            # orientation for the attention-probability output, normalized by
            # a per-partition tensor_scalar with the transposed denominators.
            for b in range(B):
                for h in range(HL):
                    hs = slice(h * DEPTH, (h + 1) * DEPTH)
                    vs = slice(h * (DEPTH + 1), (h + 1) * (DEPTH + 1))
                    recip_bh = denp.tile([128, S // 128], F32, tag="recbh")
                    for qc in range(QC):
                        q0 = b * S + qc * TCH
                        expt = exppool.tile([128, KT, TCH], BF16)
                        # dense burst of transposed-score matmuls; exp drains
                        # them to SBUF as PSUM slots free up
                        for kp in range(KT // 2):
                            sc2 = pp.tile([128, 2, TCH], F32, tag="pp")
                            for j in range(2):
                                kt = kp * 2 + j
                                nc.tensor.matmul(
                                    sc2[:, j, :],
                                    khT[hs, b * S + kt * 128:b * S + (kt + 1) * 128],
                                    qhT[hs, q0:q0 + TCH], start=True, stop=True)
                                if with_mask:
                                    mt = io.tile([128, TCH], F32, tag="x")
                                    nc.sync.dma_start(
                                        mt[:].bitcast(F32),
                                        ap["maskT"][b, kt * 128:(kt + 1) * 128,
                                                    qc * TCH:(qc + 1) * TCH])
                                    nc.vector.tensor_tensor(
                                        sc2[:, j, :], sc2[:, j, :],
                                        mt[:].bitcast(F32),
                                        op=mybir.AluOpType.add)
                            nc.scalar.activation(
                                expt[:, kp * 2:(kp + 1) * 2, :], sc2[:], Exp)
                        # attn@V accumulation burst (bf16)
                        av = pav.tile([DEPTH + 1, TCH], F32)
                        for kt in range(KT):
                            nc.tensor.matmul(av[:], vha[:, b * KT + kt, vs],
                                             expt[:, kt, :],
                                             start=(kt == 0), stop=(kt == KT - 1),
                                             skip_group_check=True)
                        den = denp.tile([1, TCH], F32)
                        nc.vector.tensor_copy(den[:], av[DEPTH:DEPTH + 1, :])
                        # transposed denominators for the natural-orientation
                        # normalize (per-partition scalars, one per q tile)
                        for qs in range(TCH // 128):
                            qt = qc * (TCH // 128) + qs
                            denT = ppn.tile([128, 1], F32, tag="ppn")
                            nc.tensor.transpose(
                                denT[:], den[0:1, qs * 128:(qs + 1) * 128],
                                identf[0:1, 0:1])
                            nc.vector.reciprocal(recip_bh[:, qt:qt + 1], denT[:])
                        # per-head normalize of attn@V before the output
                        # projection (heads mix in its contraction, so the
                        # 1/denom scale cannot be folded through it)
                        recip_row = denp.tile([1, TCH], F32, tag="rrow")
                        nc.vector.reciprocal(recip_row[:], den[:])
                        rb = denp.tile([DEPTH, TCH], F32, tag="rb")
                        nc.gpsimd.partition_broadcast(rb[:], recip_row[0:1, :])
                        avn = denp.tile([DEPTH, TCH], F32, tag="avn")
                        nc.vector.tensor_tensor(avn[:], av[0:DEPTH, :], rb[:],
                                                op=mybir.AluOpType.mult)
                        nc.scalar.activation(
                            aoT[hs, b, qc * TCH:(qc + 1) * TCH], avn[:], Copy)
                        # natural-orientation attention rows for this q chunk
                        for qs in range(TCH // 128):
                            qt = qc * (TCH // 128) + qs
                            row = rowp.tile([128, S], F32)
                            for kc in range(S // TCH):
                                sc = ppn.tile([128, TCH], F32, tag="ppn")
                                nc.tensor.matmul(
                                    sc[:],
                                    qhT[hs, b * S + qt * 128:b * S + (qt + 1) * 128],
                                    khT[hs, b * S + kc * TCH:b * S + (kc + 1) * TCH],
                                    start=True, stop=True)
                                if with_mask:
                                    mt = io.tile([128, TCH], F32, tag="x")
                                    nc.sync.dma_start(
                                        mt[:].bitcast(F32),
                                        ap["maskQ"][b, qt * 128:(qt + 1) * 128,
                                                    kc * TCH:(kc + 1) * TCH])
                                    nc.vector.tensor_tensor(
                                        sc[:], sc[:], mt[:].bitcast(F32),
                                        op=mybir.AluOpType.add)
                                nc.scalar.activation(
                                    row[:, kc * TCH:(kc + 1) * TCH], sc[:], Exp)
                            nc.vector.tensor_scalar_mul(row[:], row[:],
                                                        recip_bh[:, qt:qt + 1])
                            nc.sync.dma_start(
                                attn_d[b, h, qt * 128:(qt + 1) * 128, :], row[:])

            # ---- output projection ----
            for b in range(B):
                for tt in range(S // 128):
                    for nn in range(D // TCH):
                        ps = pp.tile([128, TCH], F32, tag="pp")
                        nc.tensor.matmul(ps[:], aoT[:, b, tt * 128:(tt + 1) * 128],
                                         wot[:, nn * TCH:(nn + 1) * TCH],
                                         start=True, stop=True)
                        osb = outp.tile([128, TCH], F32)
                        nc.vector.tensor_copy(osb[:], ps[:])
                        nc.sync.dma_start(
                            out_d[b * S + tt * 128:b * S + (tt + 1) * 128,
                                  nn * TCH:(nn + 1) * TCH], osb[:])

    nc.compile()
    return nc


def _profile_shims():
    """Optional NTFF profiling under axon (dev only, BASS_ATTN_PROFILE=1)."""
    import contextlib
    import ctypes
    import types

    so = "/opt/axon/libaxon_pjrt.so"
    try:
        lib = ctypes.CDLL(so)
        if not hasattr(lib, "axon_start_nrt_profile"):
            return False
        lib.axon_start_nrt_profile.argtypes = [ctypes.POINTER(ctypes.c_int64),
                                               ctypes.c_size_t]
        lib.axon_start_nrt_profile.restype = ctypes.c_int64
        lib.axon_stop_nrt_profile.argtypes = [ctypes.c_char_p]
        lib.axon_stop_nrt_profile.restype = ctypes.c_int64

        @contextlib.contextmanager
        def _hook(output_dir, device_ids):
            import jax
            jax.devices()
            if device_ids:
                ids = (ctypes.c_int64 * len(device_ids))(*device_ids)
                rc = lib.axon_start_nrt_profile(ids, len(device_ids))
            else:
                rc = lib.axon_start_nrt_profile(None, 0)
            if rc != 0:
                raise RuntimeError(f"axon_start_nrt_profile rc={rc}")
            try:
                yield
            finally:
                lib.axon_stop_nrt_profile(str(output_dir).encode())

        import antenv
        mod = types.ModuleType("antenv.axon_hooks")
        mod.get_axon_ntff_profile_hook = lambda: _hook
        mod.set_axon_ntff_profile_hook = lambda h: None
        sys.modules["antenv.axon_hooks"] = mod
        antenv.axon_hooks = mod
        from concourse import bass_utils
        bass_utils.upload_artifacts = lambda tmpdir: f"local://{tmpdir}"
        return True
    except Exception:
        return False


def kernel(v, k, q, mask, wq, bq, wk, bk, wv, bv, wo, bo):
    global last_exec_ns
    v, k, q = (np.asarray(x, np.float32) for x in (v, k, q))
    mask = np.asarray(mask, np.float32)
    wq, bq, wk, bk, wv, bv, wo, bo = (
        np.asarray(x, np.float32) for x in (wq, bq, wk, bk, wv, bv, wo, bo))

    scale = 1.0 / np.sqrt(np.float32(DEPTH))
    with_bias_qk = bool(bq.any() or bk.any())
    with_bias_v = bool(bv.any())
    with_mask = bool(mask.any())

    key = (with_bias_qk, with_bias_v, with_mask)
    if key not in _cache:
        _cache[key] = _build(*key)
    nc = _cache[key]

    qT = np.ascontiguousarray(q.reshape(NTOK, D).T)
    kT = np.ascontiguousarray(k.reshape(NTOK, D).T)
    vT = np.ascontiguousarray(v.reshape(NTOK, D).T)

    in_maps = []
    for c in range(NCORES):
        cols = slice(c * LOC, (c + 1) * LOC)
        m = {
            "xqT": qT, "xkT": kT, "xvT": vT,
            # fold the 1/sqrt(depth) score scale into the q projection
            "wq": np.ascontiguousarray(wq[:, cols] * scale),
            "wk": np.ascontiguousarray(wk[:, cols]),
            "wv": np.ascontiguousarray(wv[:, cols]),
            "wo": np.ascontiguousarray(wo[c * LOC:(c + 1) * LOC, :]),
        }
        if with_bias_qk:
            m["bq"] = np.ascontiguousarray((bq[cols] * scale).reshape(LOC, 1))
            m["bk"] = np.ascontiguousarray(bk[cols].reshape(LOC, 1))
        if with_bias_v:
            m["bv"] = np.ascontiguousarray(bv[cols].reshape(1, LOC))
        if with_mask:
            msc = mask[:, 0] * np.float32(-1e9)
            m["maskT"] = np.ascontiguousarray(msc.transpose(0, 2, 1))
            m["maskQ"] = np.ascontiguousarray(msc)
        in_maps.append(m)

    profile = os.environ.get("BASS_ATTN_PROFILE") == "1" and _profile_shims()
    res = run_bass_kernel_spmd(nc, in_maps, list(range(NCORES)),
                               trace=bool(profile))
    last_exec_ns = res.exec_time_ns

    out = np.zeros((NTOK, D), np.float32)
    attn = np.empty((B, H, S, S), np.float32)
    for c in range(NCORES):
        out += res.results[c]["out"]
        attn[:, c * HL:(c + 1) * HL] = res.results[c]["attn"]
    out += bo
    return out.reshape(B, S, D), attn


# revision 13
# speedup vs baseline: 1.1092x; 1.0107x over previous
"""Multi-head attention (B=2, S=2048, D=1024, H=16) on 8 TRN2 NeuronCores.

Sharding: tensor-parallel over heads — 2 heads per core. Each core computes
its head group's QKV projections, attention probabilities and attn@V, plus a
partial output projection; the host sums the 8 partial outputs and
concatenates the per-core attention blocks.

Per-core pipeline (precision-critical matmuls in fp32r; the attn@V pass in
bf16 since it only feeds the output projection):
  - qhT/khT in depth-major layout [128, B*S] straight from the projection
    matmuls; vh token-major with an extra ones-column, so the attn@V matmul
    also produces the softmax denominators for free.
  - scores are computed twice, once per orientation. Transposed [k, q]
    scores -> ACT exp (bf16, no max-subtraction: logits for this problem's
    scale are O(1), exp cannot overflow) -> attn@V accumulated in PSUM.
    Natural [q, k] scores -> ACT exp (f32) feed the attention-probability
    output: per-partition tensor_scalar normalize (denominators transposed
    via tiny PE transposes), then fully contiguous 1MB row DMAs. The dual
    score matmul is much cheaper than transposing 16.8M exp values per core.
  - attn@V is normalized per head (heads mix in the output projection's
    contraction, so the scale can't be folded through it); the output
    projection then runs in fp32r and the host sums the 8 partials.
"""

import os
import sys

import numpy as np

for _p in ("/opt/trn_rl_repo",):
    if _p not in sys.path and os.path.isdir(_p):
        sys.path.append(_p)

import concourse.bass as bass  # noqa: E402
import concourse.tile as tile  # noqa: E402
from concourse import bacc, mybir  # noqa: E402
from concourse.bass_utils import run_bass_kernel_spmd  # noqa: E402
from concourse.masks import make_identity  # noqa: E402

F32 = mybir.dt.float32
F32R = mybir.dt.float32r
BF16 = mybir.dt.bfloat16

B, S, D, H = 2, 2048, 1024, 16
DEPTH = D // H          # 64
NCORES = 8
HL = H // NCORES        # 2 heads per core
LOC = HL * DEPTH        # 128 local output dims per core
NTOK = B * S            # 4096
TCH = 512               # token chunk for projections / q chunks
KT = S // 128           # 16 k tiles per batch
QC = S // TCH           # 4 q chunks per batch

_cache = {}
last_exec_ns = None     # set when BASS_ATTN_PROFILE=1


def _build(with_bias_qk, with_bias_v, with_mask):
    nc = bacc.Bacc("TRN2", target_bir_lowering=False, debug=False,
                   num_devices=NCORES)
    ap = {}
    for name, shape, dt in [
        ("xqT", [D, NTOK], F32R), ("xkT", [D, NTOK], F32R),
        ("xvT", [D, NTOK], F32R),
        ("wq", [D, LOC], F32R), ("wk", [D, LOC], F32R), ("wv", [D, LOC], F32R),
        ("wo", [LOC, D], F32R),
    ]:
        ap[name] = nc.dram_tensor(name, shape, dt, kind="ExternalInput").ap()
    if with_bias_qk:
        ap["bq"] = nc.dram_tensor("bq", [LOC, 1], F32, kind="ExternalInput").ap()
        ap["bk"] = nc.dram_tensor("bk", [LOC, 1], F32, kind="ExternalInput").ap()
    if with_bias_v:
        ap["bv"] = nc.dram_tensor("bv", [1, LOC], F32, kind="ExternalInput").ap()
    if with_mask:
        # pre-scaled by -1e9 on host, transposed to [b, k, q]
        ap["maskT"] = nc.dram_tensor("maskT", [B, S, S], F32,
                                     kind="ExternalInput").ap()
        ap["maskQ"] = nc.dram_tensor("maskQ", [B, S, S], F32,
                                     kind="ExternalInput").ap()
    attn_d = nc.dram_tensor("attn", [B, HL, S, S], F32, kind="ExternalOutput").ap()
    out_d = nc.dram_tensor("out", [NTOK, D], F32, kind="ExternalOutput").ap()

    DCH = D // 128  # 8 contraction chunks
    Exp = mybir.ActivationFunctionType.Exp
    Copy = mybir.ActivationFunctionType.Copy
    Ident = mybir.ActivationFunctionType.Identity

    with tile.TileContext(nc) as tc:
        import contextlib
        with contextlib.ExitStack() as ctx:
            const = ctx.enter_context(tc.tile_pool(name="const", bufs=1))
            io = ctx.enter_context(tc.tile_pool(name="io", bufs=10))
            exppool = ctx.enter_context(tc.tile_pool(name="expp", bufs=2))
            rowp = ctx.enter_context(tc.tile_pool(name="rowp", bufs=3))
            denp = ctx.enter_context(tc.tile_pool(name="denp", bufs=2))
            outp = ctx.enter_context(tc.tile_pool(name="outp", bufs=3))
            pp = ctx.enter_context(tc.tile_pool(name="pp", bufs=2, space="PSUM"))
            ppn = ctx.enter_context(tc.tile_pool(name="ppn", bufs=3, space="PSUM"))
            pav = ctx.enter_context(tc.tile_pool(name="pav", bufs=1, space="PSUM"))

            identf = const.tile([32, 32], F32)
            make_identity(nc, identf[:])

            # ---- weights into SBUF ----
            wqt = const.tile([128, DCH, LOC], F32R)
            wkt = const.tile([128, DCH, LOC], F32R)
            wvt = const.tile([128, DCH, LOC], F32R)
            for t, w in ((wqt, ap["wq"]), (wkt, ap["wk"]), (wvt, ap["wv"])):
                nc.gpsimd.dma_start(t[:], w.rearrange("(c p) m -> p c m", p=128))
            wot = const.tile([LOC, D], F32R)
            nc.sync.dma_start(wot[:], ap["wo"])
            if with_bias_qk:
                bq_sb = const.tile([LOC, 1], F32)
                nc.sync.dma_start(bq_sb[:], ap["bq"])
                bk_sb = const.tile([LOC, 1], F32)
                nc.sync.dma_start(bk_sb[:], ap["bk"])
            if with_bias_v:
                bv_row = const.tile([1, LOC], F32)
                nc.sync.dma_start(bv_row[:], ap["bv"])
                bv_sb = const.tile([128, LOC], F32)
                nc.gpsimd.partition_broadcast(bv_sb[:], bv_row[0:1, :])

            # ---- persistent activations ----
            qhT = const.tile([LOC, NTOK], F32R)
            khT = const.tile([LOC, NTOK], F32R)
            vha = const.tile([128, B * KT, 2 * (DEPTH + 1)], BF16)
            aoT = const.tile([LOC, B, S], F32R)

            # ---- per-batch: project, then attend (keeps the PE dense:
            # batch b+1's DMA-paced projections overlap batch b's attention)
            for b in range(B):
              # q/k projections: dest[c, tok] += w[d, c]^T x[d, tok]
              for xname, wt, bias, dest in (
                ("xqT", wqt, "bq", qhT), ("xkT", wkt, "bk", khT),
              ):
                for tcn in range(b * (S // TCH), (b + 1) * (S // TCH)):
                    ps2 = pp.tile([128, 2, TCH], F32, tag="pp")
                    ps = ps2[:, 0, :]
                    for dc in range(DCH):
                        xt = io.tile([128, TCH], F32R, tag="x")
                        nc.gpsimd.dma_start(
                            xt[:], ap[xname][dc * 128:(dc + 1) * 128,
                                             tcn * TCH:(tcn + 1) * TCH])
                        nc.tensor.matmul(ps[:], wt[:, dc, :], xt[:],
                                         start=(dc == 0), stop=(dc == DCH - 1))
                    dslice = dest[:, tcn * TCH:(tcn + 1) * TCH]
                    if with_bias_qk:
                        nc.scalar.activation(dslice, ps[:], Ident,
                                             bias=(bq_sb if bias == "bq" else bk_sb)[:])
                    else:
                        nc.scalar.activation(dslice, ps[:], Copy)

              # v projection (token-major) + ones columns
              for tt in range(b * (S // 128), (b + 1) * (S // 128)):
                ps2 = pp.tile([128, 2, LOC], F32, tag="pp")
                ps = ps2[:, 0, :]
                for dc in range(DCH):
                    vt = io.tile([128, 128], F32R, tag="vx")
                    nc.sync.dma_start(
                        vt[:], ap["xvT"][dc * 128:(dc + 1) * 128,
                                         tt * 128:(tt + 1) * 128])
                    nc.tensor.matmul(ps[:], vt[:], wvt[:, dc, :],
                                     start=(dc == 0), stop=(dc == DCH - 1))
                if with_bias_v:
                    nc.vector.tensor_tensor(ps[:], ps[:], bv_sb[:],
                                            op=mybir.AluOpType.add)
                for h in range(HL):
                    nc.scalar.activation(
                        vha[:, tt, h * (DEPTH + 1):h * (DEPTH + 1) + DEPTH],
                        ps[:, h * DEPTH:(h + 1) * DEPTH], Copy)
              if b == 0:
                for h in range(HL):
                    nc.vector.memset(vha[:, :, h * (DEPTH + 1) + DEPTH], 1.0)

              # attention: stream 1 computes transposed scores [k, q] ->
              # exp (bf16) -> attn@V (+ denominators via the vh ones-column);
              # stream 2 recomputes scores in natural [q, k] orientation for
              # the attention-probability output.
              if True:
                for h in range(HL):
                    hs = slice(h * DEPTH, (h + 1) * DEPTH)
                    vs = slice(h * (DEPTH + 1), (h + 1) * (DEPTH + 1))
                    recip_bh = denp.tile([128, S // 128], F32, tag="recbh")
                    for qc in range(QC):
                        q0 = b * S + qc * TCH
                        expt = exppool.tile([128, KT, TCH], BF16)
                        # dense burst of transposed-score matmuls; exp drains
                        # them to SBUF as PSUM slots free up
                        for kp in range(KT // 2):
                            sc2 = pp.tile([128, 2, TCH], F32, tag="pp")
                            for j in range(2):
                                kt = kp * 2 + j
                                nc.tensor.matmul(
                                    sc2[:, j, :],
                                    khT[hs, b * S + kt * 128:b * S + (kt + 1) * 128],
                                    qhT[hs, q0:q0 + TCH], start=True, stop=True)
                                if with_mask:
                                    mt = io.tile([128, TCH], F32, tag="x")
                                    nc.sync.dma_start(
                                        mt[:].bitcast(F32),
                                        ap["maskT"][b, kt * 128:(kt + 1) * 128,
                                                    qc * TCH:(qc + 1) * TCH])
                                    nc.vector.tensor_tensor(
                                        sc2[:, j, :], sc2[:, j, :],
                                        mt[:].bitcast(F32),
                                        op=mybir.AluOpType.add)
                            nc.scalar.activation(
                                expt[:, kp * 2:(kp + 1) * 2, :], sc2[:], Exp)
                        # attn@V accumulation burst (bf16)
                        av = pav.tile([DEPTH + 1, TCH], F32)
                        for kt in range(KT):
                            nc.tensor.matmul(av[:], vha[:, b * KT + kt, vs],
                                             expt[:, kt, :],
                                             start=(kt == 0), stop=(kt == KT - 1),
                                             skip_group_check=True)
                        den = denp.tile([1, TCH], F32)
                        nc.vector.tensor_copy(den[:], av[DEPTH:DEPTH + 1, :])
                        # transposed denominators for the natural-orientation
                        # normalize (per-partition scalars, one per q tile)
                        for qs in range(TCH // 128):
                            qt = qc * (TCH // 128) + qs
                            denT = ppn.tile([128, 1], F32, tag="ppn")
                            nc.tensor.transpose(
                                denT[:], den[0:1, qs * 128:(qs + 1) * 128],
                                identf[0:1, 0:1])
                            nc.vector.reciprocal(recip_bh[:, qt:qt + 1], denT[:])
                        # per-head normalize of attn@V before the output
                        # projection (heads mix in its contraction, so the
                        # 1/denom scale cannot be folded through it)
                        recip_row = denp.tile([1, TCH], F32, tag="rrow")
                        nc.vector.reciprocal(recip_row[:], den[:])
                        rb = denp.tile([DEPTH, TCH], F32, tag="rb")
                        nc.gpsimd.partition_broadcast(rb[:], recip_row[0:1, :])
                        avn = denp.tile([DEPTH, TCH], F32, tag="avn")
                        nc.vector.tensor_tensor(avn[:], av[0:DEPTH, :], rb[:],
                                                op=mybir.AluOpType.mult)
                        nc.scalar.activation(
                            aoT[hs, b, qc * TCH:(qc + 1) * TCH], avn[:], Copy)
                        # natural-orientation attention rows for this q chunk
                        for qs in range(TCH // 128):
                            qt = qc * (TCH // 128) + qs
                            row = rowp.tile([128, S], F32)
                            for kc in range(S // TCH):
                                sc = ppn.tile([128, TCH], F32, tag="ppn")
                                nc.tensor.matmul(
                                    sc[:],
                                    qhT[hs, b * S + qt * 128:b * S + (qt + 1) * 128],
                                    khT[hs, b * S + kc * TCH:b * S + (kc + 1) * TCH],
                                    start=True, stop=True)
                                if with_mask:
                                    mt = io.tile([128, TCH], F32, tag="x")
                                    nc.sync.dma_start(
                                        mt[:].bitcast(F32),
                                        ap["maskQ"][b, qt * 128:(qt + 1) * 128,
                                                    kc * TCH:(kc + 1) * TCH])
                                    nc.vector.tensor_tensor(
                                        sc[:], sc[:], mt[:].bitcast(F32),
                                        op=mybir.AluOpType.add)
                                nc.scalar.activation(
                                    row[:, kc * TCH:(kc + 1) * TCH], sc[:], Exp)
                            nc.vector.tensor_scalar_mul(row[:], row[:],
                                                        recip_bh[:, qt:qt + 1])
                            nc.sync.dma_start(
                                attn_d[b, h, qt * 128:(qt + 1) * 128, :], row[:])

              # output projection
              if True:
                for tt in range(S // 128):
                    for nn in range(D // TCH):
                        ps = pp.tile([128, TCH], F32, tag="pp")
                        nc.tensor.matmul(ps[:], aoT[:, b, tt * 128:(tt + 1) * 128],
                                         wot[:, nn * TCH:(nn + 1) * TCH],
                                         start=True, stop=True)
                        osb = outp.tile([128, TCH], F32)
                        nc.vector.tensor_copy(osb[:], ps[:])
                        nc.sync.dma_start(
                            out_d[b * S + tt * 128:b * S + (tt + 1) * 128,
                                  nn * TCH:(nn + 1) * TCH], osb[:])

    nc.compile()
    return nc


def _profile_shims():
    """Optional NTFF profiling under axon (dev only, BASS_ATTN_PROFILE=1)."""
    import contextlib
    import ctypes
    import types

    so = "/opt/axon/libaxon_pjrt.so"
    try:
        lib = ctypes.CDLL(so)
        if not hasattr(lib, "axon_start_nrt_profile"):
            return False
        lib.axon_start_nrt_profile.argtypes = [ctypes.POINTER(ctypes.c_int64),
                                               ctypes.c_size_t]
        lib.axon_start_nrt_profile.restype = ctypes.c_int64
        lib.axon_stop_nrt_profile.argtypes = [ctypes.c_char_p]
        lib.axon_stop_nrt_profile.restype = ctypes.c_int64

        @contextlib.contextmanager
        def _hook(output_dir, device_ids):
            import jax
            jax.devices()
            if device_ids:
                ids = (ctypes.c_int64 * len(device_ids))(*device_ids)
                rc = lib.axon_start_nrt_profile(ids, len(device_ids))
            else:
                rc = lib.axon_start_nrt_profile(None, 0)
            if rc != 0:
                raise RuntimeError(f"axon_start_nrt_profile rc={rc}")
            try:
                yield
            finally:
                lib.axon_stop_nrt_profile(str(output_dir).encode())

        import antenv
        mod = types.ModuleType("antenv.axon_hooks")
        mod.get_axon_ntff_profile_hook = lambda: _hook
        mod.set_axon_ntff_profile_hook = lambda h: None
        sys.modules["antenv.axon_hooks"] = mod
        antenv.axon_hooks = mod
        from concourse import bass_utils
        bass_utils.upload_artifacts = lambda tmpdir: f"local://{tmpdir}"
        return True
    except Exception:
        return False


def kernel(v, k, q, mask, wq, bq, wk, bk, wv, bv, wo, bo):
    global last_exec_ns
    v, k, q = (np.asarray(x, np.float32) for x in (v, k, q))
    mask = np.asarray(mask, np.float32)
    wq, bq, wk, bk, wv, bv, wo, bo = (
        np.asarray(x, np.float32) for x in (wq, bq, wk, bk, wv, bv, wo, bo))

    scale = 1.0 / np.sqrt(np.float32(DEPTH))
    with_bias_qk = bool(bq.any() or bk.any())
    with_bias_v = bool(bv.any())
    with_mask = bool(mask.any())

    key = (with_bias_qk, with_bias_v, with_mask)
    if key not in _cache:
        _cache[key] = _build(*key)
    nc = _cache[key]

    qT = np.ascontiguousarray(q.reshape(NTOK, D).T)
    kT = np.ascontiguousarray(k.reshape(NTOK, D).T)
    vT = np.ascontiguousarray(v.reshape(NTOK, D).T)

    in_maps = []
    for c in range(NCORES):
        cols = slice(c * LOC, (c + 1) * LOC)
        m = {
            "xqT": qT, "xkT": kT, "xvT": vT,
            # fold the 1/sqrt(depth) score scale into the q projection
            "wq": np.ascontiguousarray(wq[:, cols] * scale),
            "wk": np.ascontiguousarray(wk[:, cols]),
            "wv": np.ascontiguousarray(wv[:, cols]),
            "wo": np.ascontiguousarray(wo[c * LOC:(c + 1) * LOC, :]),
        }
        if with_bias_qk:
            m["bq"] = np.ascontiguousarray((bq[cols] * scale).reshape(LOC, 1))
            m["bk"] = np.ascontiguousarray(bk[cols].reshape(LOC, 1))
        if with_bias_v:
            m["bv"] = np.ascontiguousarray(bv[cols].reshape(1, LOC))
        if with_mask:
            msc = mask[:, 0] * np.float32(-1e9)
            m["maskT"] = np.ascontiguousarray(msc.transpose(0, 2, 1))
            m["maskQ"] = np.ascontiguousarray(msc)
        in_maps.append(m)

    profile = os.environ.get("BASS_ATTN_PROFILE") == "1" and _profile_shims()
    res = run_bass_kernel_spmd(nc, in_maps, list(range(NCORES)),
                               trace=bool(profile))
    last_exec_ns = res.exec_time_ns

    out = np.zeros((NTOK, D), np.float32)
    attn = np.empty((B, H, S, S), np.float32)
    for c in range(NCORES):
        out += res.results[c]["out"]
        attn[:, c * HL:(c + 1) * HL] = res.results[c]["attn"]
    out += bo
    return out.reshape(B, S, D), attn
